# revision 27
# baseline (speedup 1.0000x reference)
"""Grouped-Query Attention (B=2, T=2048, H=2048, 16 q-heads, 4 kv-heads, d=128,
causal) on 8 Trainium2 NeuronCores.

Sharding: core c = (batch b, kv-group g) with b = c // 4, g = c % 4.
Each core handles one batch element, one kv head, and its 4 q heads:
  - Q/K/V projections for its slice (tensor-parallel over heads)
  - causal attention for 4 q heads against the shared K/V head
  - partial o_proj (row-parallel): out_partial = O_heads @ w_o[:, cols].T
Host sums the 4 per-batch partials (the row-parallel all-reduce) and stacks.

Device layouts (chosen so no transposes are ever needed on-chip):
  QT, KT: [d=128, T]  (projection computed directly transposed)
  V:      [T-tile=128, d]
  scores: computed directly transposed as ST [k, q] via lhsT=KT_j, rhs=QT
  P = exp(ST/sqrt(d)) stays [k, q] and feeds PV as rhs -> OT [d, q] which is
  exactly the lhsT the o_proj needs. Row sums of P (softmax denominator) are
  computed broadcast via an all-ones [128,128] stationary matmul.
All matmul inputs bf16, PSUM accumulation fp32, softmax in fp32.

v3 scheduling:
  - PE program order interleaves phases: proj(0), attn(0), proj(1), oproj(0),
    attn(1), proj(2), oproj(1), attn(2), proj(3), oproj(2), attn(3), oproj(3).
    Attention needs no DMA, so it covers the windows where xT/weight streams
    are still in flight; projections cover attention's norm-chain bubbles.
  - Input DMA split across both HWDGE queues: weights (wq chunks, wk, wv, msk,
    wo) on the Activation queue, xT (blocks 0/1 chunk-granular, 2/3 as single
    descriptors) on the SP queue, ordered by first-use time.
  - Diagonal causal trim: the 4 diagonal key tiles of each (qc, h) group only
    compute the live query columns (scores/exp/mask/PV/row-sum all narrowed).
  - Scores land in [128, 2, 512] PSUM pairs; one Act exp covers both j-tiles.
  - Softmax normalization via one fast-approx DVE reciprocal + one DVE
    multiply reading OT straight from PSUM.
  - o_proj stage copies run on the (otherwise idle) Pool engine.
"""

import numpy as np
import ml_dtypes
from contextlib import ExitStack

import concourse.bass as bass
import concourse.mybir as mybir
import concourse.tile as tile
from concourse.bass_utils import run_bass_kernel_spmd

# ---------------------------------------------------------------------------
# Workaround for this compiler build's per-instruction sync-wait-slot limit
# (walrus setupSyncWait rejects >2 waits on an instruction). Post-process the
# serialized BIR: any instruction carrying more than 2 sem waits gets the
# excess moved onto injected same-engine Drain instructions placed directly
# before it (same queue, program order => identical semantics).
import json as _json

_WAIT_LIMITS = {}
_WAIT_LIMIT_DEFAULT = 1
_orig_to_json_bytes = bass.Bass.to_json_bytes


def _split_waits_json(bj: bytes) -> bytes:
    m = _json.loads(bj)
    ctr = 0
    changed = False
    for f in m["functions"]:
        for blk in f["blocks"]:
            out = []
            for inst in blk["instructions"]:
                si = inst.get("sync_info") or {}
                w = si.get("on_wait") or []
                lim = _WAIT_LIMITS.get(inst.get("opcode"), _WAIT_LIMIT_DEFAULT)
                if len(w) > lim:
                    changed = True
                    extra, keep = w[:-lim], w[-lim:]
                    si["on_wait"] = keep
                    for i in range(0, len(extra), 1):
                        ctr += 1
                        out.append({
                            "debug": inst.get("debug", 0),
                            "engine": inst["engine"],
                            "ins": [],
                            "is_reset_sema": False,
                            "name": f"I-wsplit-{ctr}",
                            "opcode": "Drain",
                            "outs": [],
                            "sync_info": {
                                "on_update": [],
                                "on_wait": extra[i:i + 1],
                            },
                        })
                out.append(inst)
            if changed:
                blk["instructions"] = out
    if not changed:
        return bj
    return _json.dumps(m).encode()


def _to_json_bytes_patched(self, *a, **k):
    return _split_waits_json(_orig_to_json_bytes(self, *a, **k))


bass.Bass.to_json_bytes = _to_json_bytes_patched
# ---------------------------------------------------------------------------

HIDDEN = 2048
N_HEADS = 16
N_KV = 4
HD = 128
B, T = 2, 2048
G = N_HEADS // N_KV          # q heads per core = 4
HC = HIDDEN // 128           # contraction chunks = 16
NCORES = 8
SCALE = HD ** -0.5

BF16 = mybir.dt.bfloat16
F32 = mybir.dt.float32

_CACHE = {}
LAST_RESULTS = None


def _build_program():
    nc = bass.Bass("TRN2")
    xT = nc.dram_tensor("xT", [HIDDEN, T], BF16, kind="ExternalInput")
    wq = nc.dram_tensor("wq", [HIDDEN, G * HD], BF16, kind="ExternalInput")
    wk = nc.dram_tensor("wk", [HIDDEN, HD], BF16, kind="ExternalInput")
    wv = nc.dram_tensor("wv", [HIDDEN, HD], BF16, kind="ExternalInput")
    wo = nc.dram_tensor("wo", [G * HD, HIDDEN], BF16, kind="ExternalInput")
    msk = nc.dram_tensor("msk", [128, G, 512], BF16, kind="ExternalInput")
    out = nc.dram_tensor("out", [T, HIDDEN], BF16, kind="ExternalOutput")

    xTv = xT.rearrange("(c p) t -> p c t", p=128)
    wqv = wq.rearrange("(c p) m -> p c m", p=128)
    wkv = wk.rearrange("(c p) d -> p c d", p=128)
    wvv = wv.rearrange("(c p) d -> p c d", p=128)
    wov = wo.rearrange("(h p) e -> p h e", p=128)

    EXP = mybir.ActivationFunctionType.Exp

    with tile.TileContext(nc) as tc, ExitStack() as ctx:
        sing = ctx.enter_context(tc.tile_pool(name="sing", bufs=1))
        ptp = ctx.enter_context(tc.tile_pool(name="ptp", bufs=4))
        vecp = ctx.enter_context(tc.tile_pool(name="vecp", bufs=2))
        otnp = ctx.enter_context(tc.tile_pool(name="otnp", bufs=8))
        outp = ctx.enter_context(tc.tile_pool(name="outp", bufs=3))
        psum = ctx.enter_context(tc.tile_pool(name="psum", bufs=2, space="PSUM"))

        xT_sb = sing.tile([128, HC, T], BF16)
        wq_sb = sing.tile([128, HC, G * HD], BF16)
        wk_sb = sing.tile([128, HC, HD], BF16)
        wv_sb = sing.tile([128, HC, HD], BF16)
        wo_sb = sing.tile([128, G, HIDDEN], BF16)
        msk_sb = sing.tile([128, G, 512], BF16)
        ones_sb = sing.tile([128, 128], BF16)
        qt_sb = sing.tile([128, G, T], BF16)
        kt_sb = sing.tile([128, T], BF16)
        v_sb = sing.tile([128, HC, HD], BF16)

        nc.vector.memset(ones_sb, 1.0)

        # ---- input DMA: weights on the Act HWDGE queue, xT block 0 on the SP
        # queue, xT block 1 on the Pool SWDGE queue (third parallel channel),
        # all ordered by first-use time. Progressive batch sizes: fine granules
        # first so compute starts early, then 4-8 chunk batches (hundreds of
        # descriptors per instruction) to keep the hardware queues deep.
        # Act queue: wv/wk first (vproj/kproj run first), then wq, msk, wo.
        nc.scalar.dma_start(out=wv_sb[:, 0:8, :], in_=wvv[:, 0:8, :])
        nc.scalar.dma_start(out=wv_sb[:, 8:16, :], in_=wvv[:, 8:16, :])
        nc.scalar.dma_start(out=wk_sb, in_=wkv)
        for c0, c1 in ((0, 2), (2, 4), (4, 8), (8, 16)):
            nc.scalar.dma_start(out=wq_sb[:, c0:c1, :], in_=wqv[:, c0:c1, :])
        nc.scalar.dma_start(out=msk_sb, in_=msk[:, :, :])
        nc.scalar.dma_start(out=wo_sb, in_=wov)
        # SP queue: xT block 0 progressively; block 1's first group squeezes in
        # before block 0's tail (slack in proj(0)'s consumption) so proj(1)
        # never waits; blocks 2/3 as single deep-queue instructions.
        for c0, c1 in ((0, 1), (1, 2), (2, 4), (4, 8)):
            nc.sync.dma_start(out=xT_sb[:, c0:c1, 0:512],
                              in_=xTv[:, c0:c1, 0:512])
        nc.sync.dma_start(out=xT_sb[:, 0:4, 512:1024], in_=xTv[:, 0:4, 512:1024])
        nc.sync.dma_start(out=xT_sb[:, 8:16, 0:512], in_=xTv[:, 8:16, 0:512])
        for c4 in range(4, HC, 4):
            nc.sync.dma_start(out=xT_sb[:, c4:c4 + 4, 512:1024],
                              in_=xTv[:, c4:c4 + 4, 512:1024])
        nc.sync.dma_start(out=xT_sb[:, :, 1024:1536], in_=xTv[:, :, 1024:1536])
        nc.sync.dma_start(out=xT_sb[:, :, 1536:2048], in_=xTv[:, :, 1536:2048])

        otns = {}

        def proj(t4):
            # v, k first, q heads last: the next phase's first PSUM allocation
            # on the shared tag then waits on an early copy, not the last one.
            tsl = slice(t4 * 512, (t4 + 1) * 512)
            for tt in range(4 * t4, 4 * t4 + 4):
                vp = psum.tile([128, HD], F32, tag="ot", bufs=2, name=f"vp_{tt}")
                for c in range(HC):
                    nc.tensor.matmul(
                        vp,
                        lhsT=xT_sb[:, c, tt * 128:(tt + 1) * 128],
                        rhs=wv_sb[:, c, :],
                        start=(c == 0), stop=(c == HC - 1),
                    )
                nc.vector.tensor_copy(v_sb[:, tt, :], vp)
            kp = psum.tile([128, 512], F32, tag="ot", bufs=2, name=f"kp_{t4}")
            for c in range(HC):
                nc.tensor.matmul(
                    kp,
                    lhsT=wk_sb[:, c, :],
                    rhs=xT_sb[:, c, tsl],
                    start=(c == 0), stop=(c == HC - 1),
                )
            nc.vector.tensor_copy(kt_sb[:, tsl], kp)
            for h in range(G):
                qp = psum.tile([128, 512], F32, tag="ot", bufs=2,
                               name=f"qp_{h}_{t4}")
                for c in range(HC):
                    nc.tensor.matmul(
                        qp,
                        lhsT=wq_sb[:, c, h * HD:(h + 1) * HD],
                        rhs=xT_sb[:, c, tsl],
                        start=(c == 0), stop=(c == HC - 1),
                    )
                nc.vector.tensor_copy(qt_sb[:, h, tsl], qp)

        def attn(qc):
            # Flat software-pipelined stream over (head, key-tile-pair):
            # scores for pair i+1 issue on the PE while the Act engine exps
            # pair i and the PE then runs PV/row-sum of pair i. Uniform across
            # head boundaries, so the last diagonal exp of a head no longer
            # stalls the next head's scores on st-buffer reuse.
            njt = 4 * qc + 4
            items = []
            for h in range(G):
                for p in range(njt // 2):
                    items.append((h, 2 * p, 2 * p == njt - 2))

            def emit_scores(it):
                h, j0, _ = it
                st = psum.tile([128, 1024], F32, tag="st", bufs=2,
                               name=f"st_{qc}_{h}_{j0}")
                pt = ptp.tile([128, 1024], BF16, tag="pt", bufs=8,
                              name=f"pt_{qc}_{h}_{j0}")
                lor = []
                for jj in range(2):
                    j = j0 + jj
                    jr = j - 4 * qc      # >= 0 on the diagonal block
                    lo = jr * 128 if jr > 0 else 0
                    lor.append((j, jr, lo))
                    nc.tensor.matmul(
                        st[:, jj * 512 + lo:(jj + 1) * 512],
                        lhsT=kt_sb[:, j * 128:(j + 1) * 128],
                        rhs=qt_sb[:, h, qc * 512 + lo:(qc + 1) * 512],
                        start=True, stop=True,
                    )
                lo0 = lor[0][2]
                # one exp covering both written halves; the [512, 512+lo1)
                # gap is stale-but-bounded PSUM whose pt image is never read.
                nc.scalar.activation(pt[:, lo0:], st[:, lo0:],
                                     EXP, scale=float(SCALE))
                for jj, (j, jr, lo) in enumerate(lor):
                    if jr >= 0:
                        nc.vector.tensor_mul(
                            pt[:, jj * 512 + lo:(jj + 1) * 512],
                            pt[:, jj * 512 + lo:(jj + 1) * 512],
                            msk_sb[:, jr, lo:])
                return (it, pt, lor)

            def emit_pvls(sc):
                (h, j0, last), pt, lor = sc
                if j0 == 0:
                    state[h] = (
                        psum.tile([128, 512], F32, tag="ot", bufs=2,
                                  name=f"ot_{qc}_{h}"),
                        psum.tile([128, 512], F32, tag="lsd", bufs=2,
                                  name=f"ls_{qc}_{h}"),
                    )
                ot, ls = state[h]
                for jj, (j, jr, lo) in enumerate(lor):
                    nc.tensor.matmul(ot[:, lo:], lhsT=v_sb[:, j, :],
                                     rhs=pt[:, jj * 512 + lo:(jj + 1) * 512],
                                     start=(j == 0), stop=(j == njt - 1),
                                     skip_group_check=True)
                for jj, (j, jr, lo) in enumerate(lor):
                    nc.tensor.matmul(ls[:, lo:], lhsT=ones_sb,
                                     rhs=pt[:, jj * 512 + lo:(jj + 1) * 512],
                                     start=(j == 0), stop=(j == njt - 1),
                                     skip_group_check=True)
                if last:
                    lnl = vecp.tile([128, 512], F32, tag="lnl", bufs=2,
                                    name=f"lnl_{qc}_{h}")
                    nc.scalar.activation(lnl, ls,
                                         mybir.ActivationFunctionType.Ln)
                    rec = vecp.tile([128, 512], F32, tag="rec", bufs=2,
                                    name=f"rec_{qc}_{h}")
                    nc.scalar.activation(rec, lnl, EXP, scale=-1.0)
                    otn = otnp.tile([128, 512], BF16, tag="otn", bufs=8,
                                    name=f"otn_{qc}_{h}")
                    nc.vector.tensor_mul(otn, ot, rec)
                    otns[h] = otn

            state = {}
            prev = None
            for it in items:
                sc = emit_scores(it)
                if prev is not None:
                    emit_pvls(prev)
                prev = sc
            emit_pvls(prev)

        def oproj(qc, last=False):
            for tt in range(4):
                stage = outp.tile([128, HIDDEN], BF16, tag="stage", bufs=3,
                                  name=f"stage_{qc}_{tt}")
                final = last
                # last phase: per-ec DMAs alternating across both HWDGE
                # queues so the final 2MB drains at double bandwidth (the Act
                # queue is idle then; earlier phases keep it clear for exps)
                for ec in range(4):
                    op = psum.tile([128, 512], F32, tag="ot", bufs=2,
                                   name=f"op_{qc}_{tt}_{ec}")
                    for h in range(G):
                        nc.tensor.matmul(
                            op,
                            lhsT=otns[h][:, tt * 128:(tt + 1) * 128],
                            rhs=wo_sb[:, h, ec * 512:(ec + 1) * 512],
                            start=(h == 0), stop=(h == G - 1),
                        )
                    nc.scalar.copy(stage[:, ec * 512:(ec + 1) * 512], op)
                    if final:
                        # last output tiles: per-ec DMA to shorten the tail
                        r0 = qc * 512 + tt * 128
                        eng = nc.scalar if ec % 2 == 1 else nc.sync
                        eng.dma_start(
                            out=out[r0:r0 + 128, ec * 512:(ec + 1) * 512],
                            in_=stage[:, ec * 512:(ec + 1) * 512])
                if not final:
                    r0 = qc * 512 + tt * 128
                    nc.sync.dma_start(out=out[r0:r0 + 128, :], in_=stage)

        proj(0)
        attn(0)
        proj(1)
        oproj(0)
        attn(1)
        proj(2)
        oproj(1)
        attn(2)
        proj(3)
        oproj(2)
        attn(3)
        oproj(3, last=True)
    return nc


def _masks():
    kl = np.arange(128)[:, None, None]
    jj = np.arange(G)[None, :, None]
    ql = np.arange(512)[None, None, :]
    return (128 * jj + kl <= ql).astype(ml_dtypes.bfloat16)


def kernel(x, w_q, w_kv, w_o):
    global LAST_RESULTS
    if "nc" not in _CACHE:
        _CACHE["nc"] = _build_program()
        _CACHE["msk"] = _masks()
    nc = _CACHE["nc"]
    bf = ml_dtypes.bfloat16
    x = np.asarray(x, dtype=np.float32)
    w_q = np.asarray(w_q, dtype=np.float32)
    w_kv = np.asarray(w_kv, dtype=np.float32)
    w_o = np.asarray(w_o, dtype=np.float32)

    in_maps = []
    for c in range(NCORES):
        b, g = c // 4, c % 4
        in_maps.append({
            "xT": np.ascontiguousarray(x[b].T).astype(bf),
            "wq": np.ascontiguousarray(w_q[512 * g:512 * (g + 1), :].T).astype(bf),
            "wk": np.ascontiguousarray(w_kv[128 * g:128 * (g + 1), :].T).astype(bf),
            "wv": np.ascontiguousarray(
                w_kv[512 + 128 * g:512 + 128 * (g + 1), :].T).astype(bf),
            "wo": np.ascontiguousarray(w_o[:, 512 * g:512 * (g + 1)].T).astype(bf),
            "msk": _CACHE["msk"],
        })

    res = run_bass_kernel_spmd(nc, in_maps, core_ids=list(range(NCORES)))
    LAST_RESULTS = res
    outs = res.results
    o = [outs[c]["out"].astype(np.float32) for c in range(NCORES)]
    out = np.stack([o[0] + o[1] + o[2] + o[3], o[4] + o[5] + o[6] + o[7]])
    return out


# revision 33
# speedup vs baseline: 1.0772x; 1.0772x over previous
"""Grouped-Query Attention (B=2, T=2048, H=2048, 16 q-heads, 4 kv-heads, d=128,
causal) on 8 Trainium2 NeuronCores.

Sharding: core c = (batch b, kv-group g) with b = c // 4, g = c % 4.
Each core handles one batch element, one kv head, and its 4 q heads:
  - Q/K/V projections for its slice (tensor-parallel over heads)
  - causal attention for 4 q heads against the shared K/V head
  - partial o_proj (row-parallel): out_partial = O_heads @ w_o[:, cols].T
Host sums the 4 per-batch partials (the row-parallel all-reduce) and stacks.

Device layouts (chosen so no transposes are ever needed on-chip):
  QT, KT: [d=128, T]  (projection computed directly transposed)
  V:      [T-tile=128, d]
  scores: computed directly transposed as ST [k, q] via lhsT=KT_j, rhs=QT
  P = exp(ST/sqrt(d)) stays [k, q] and feeds PV as rhs -> OT [d, q] which is
  exactly the lhsT the o_proj needs. Row sums of P (softmax denominator) are
  computed broadcast via an all-ones [128,128] stationary matmul.
All matmul inputs bf16, PSUM accumulation fp32, softmax in fp32.

v3 scheduling:
  - PE program order interleaves phases: proj(0), attn(0), proj(1), oproj(0),
    attn(1), proj(2), oproj(1), attn(2), proj(3), oproj(2), attn(3), oproj(3).
    Attention needs no DMA, so it covers the windows where xT/weight streams
    are still in flight; projections cover attention's norm-chain bubbles.
  - Input DMA split across both HWDGE queues: weights (wq chunks, wk, wv, msk,
    wo) on the Activation queue, xT (blocks 0/1 chunk-granular, 2/3 as single
    descriptors) on the SP queue, ordered by first-use time.
  - Diagonal causal trim: the 4 diagonal key tiles of each (qc, h) group only
    compute the live query columns (scores/exp/mask/PV/row-sum all narrowed).
  - Scores land in [128, 2, 512] PSUM pairs; one Act exp covers both j-tiles.
  - Softmax normalization via one fast-approx DVE reciprocal + one DVE
    multiply reading OT straight from PSUM.
  - o_proj stage copies run on the (otherwise idle) Pool engine.
"""

import numpy as np
import ml_dtypes
from contextlib import ExitStack

import concourse.bass as bass
import concourse.mybir as mybir
import concourse.tile as tile
from concourse.bass_utils import run_bass_kernel_spmd

# ---------------------------------------------------------------------------
# Workaround for this compiler build's per-instruction sync-wait-slot limit
# (walrus setupSyncWait rejects >2 waits on an instruction). Post-process the
# serialized BIR: any instruction carrying more than 2 sem waits gets the
# excess moved onto injected same-engine Drain instructions placed directly
# before it (same queue, program order => identical semantics).
import json as _json

_WAIT_LIMITS = {}
_WAIT_LIMIT_DEFAULT = 1
_orig_to_json_bytes = bass.Bass.to_json_bytes


def _split_waits_json(bj: bytes) -> bytes:
    m = _json.loads(bj)
    ctr = 0
    changed = False
    for f in m["functions"]:
        for blk in f["blocks"]:
            out = []
            for inst in blk["instructions"]:
                si = inst.get("sync_info") or {}
                w = si.get("on_wait") or []
                lim = _WAIT_LIMITS.get(inst.get("opcode"), _WAIT_LIMIT_DEFAULT)
                if len(w) > lim:
                    changed = True
                    extra, keep = w[:-lim], w[-lim:]
                    si["on_wait"] = keep
                    for i in range(0, len(extra), 1):
                        ctr += 1
                        out.append({
                            "debug": inst.get("debug", 0),
                            "engine": inst["engine"],
                            "ins": [],
                            "is_reset_sema": False,
                            "name": f"I-wsplit-{ctr}",
                            "opcode": "Drain",
                            "outs": [],
                            "sync_info": {
                                "on_update": [],
                                "on_wait": extra[i:i + 1],
                            },
                        })
                out.append(inst)
            if changed:
                blk["instructions"] = out
    if not changed:
        return bj
    return _json.dumps(m).encode()


def _to_json_bytes_patched(self, *a, **k):
    return _split_waits_json(_orig_to_json_bytes(self, *a, **k))


bass.Bass.to_json_bytes = _to_json_bytes_patched
# ---------------------------------------------------------------------------

HIDDEN = 2048
N_HEADS = 16
N_KV = 4
HD = 128
B, T = 2, 2048
G = N_HEADS // N_KV          # q heads per core = 4
HC = HIDDEN // 128           # contraction chunks = 16
NCORES = 8
SCALE = HD ** -0.5

BF16 = mybir.dt.bfloat16
F32 = mybir.dt.float32

_CACHE = {}
LAST_RESULTS = None


def _build_program():
    nc = bass.Bass("TRN2")
    xT = nc.dram_tensor("xT", [HIDDEN, T], BF16, kind="ExternalInput")
    wq = nc.dram_tensor("wq", [HIDDEN, G * HD], BF16, kind="ExternalInput")
    wk = nc.dram_tensor("wk", [HIDDEN, HD], BF16, kind="ExternalInput")
    wv = nc.dram_tensor("wv", [HIDDEN, HD], BF16, kind="ExternalInput")
    wo = nc.dram_tensor("wo", [G * HD, HIDDEN], BF16, kind="ExternalInput")
    msk = nc.dram_tensor("msk", [128, G, 512], BF16, kind="ExternalInput")
    out = nc.dram_tensor("out", [T, HIDDEN], BF16, kind="ExternalOutput")

    xTv = xT.rearrange("(c p) t -> p c t", p=128)
    wqv = wq.rearrange("(c p) m -> p c m", p=128)
    wkv = wk.rearrange("(c p) d -> p c d", p=128)
    wvv = wv.rearrange("(c p) d -> p c d", p=128)
    wov = wo.rearrange("(h p) e -> p h e", p=128)

    EXP = mybir.ActivationFunctionType.Exp

    with tile.TileContext(nc) as tc, ExitStack() as ctx:
        sing = ctx.enter_context(tc.tile_pool(name="sing", bufs=1))
        ptp = ctx.enter_context(tc.tile_pool(name="ptp", bufs=4))
        vecp = ctx.enter_context(tc.tile_pool(name="vecp", bufs=2))
        otnp = ctx.enter_context(tc.tile_pool(name="otnp", bufs=8))
        outp = ctx.enter_context(tc.tile_pool(name="outp", bufs=3))
        psum = ctx.enter_context(tc.tile_pool(name="psum", bufs=2, space="PSUM"))

        xT_sb = sing.tile([128, HC, T], BF16)
        wq_sb = sing.tile([128, HC, G * HD], BF16)
        wk_sb = sing.tile([128, HC, HD], BF16)
        wv_sb = sing.tile([128, HC, HD], BF16)
        wo_sb = sing.tile([128, G, HIDDEN], BF16)
        msk_sb = sing.tile([128, G, 512], BF16)
        ones_sb = sing.tile([128, 128], BF16)
        qt_sb = sing.tile([128, G, T], BF16)
        kt_sb = sing.tile([128, T], BF16)
        v_sb = sing.tile([128, HC, HD], BF16)

        nc.vector.memset(ones_sb, 1.0)

        # ---- input DMA: weights on the Act HWDGE queue, xT block 0 on the SP
        # queue, xT block 1 on the Pool SWDGE queue (third parallel channel),
        # all ordered by first-use time. Progressive batch sizes: fine granules
        # first so compute starts early, then 4-8 chunk batches (hundreds of
        # descriptors per instruction) to keep the hardware queues deep.
        # Act queue: wv/wk first (vproj/kproj run first), then wq, msk, wo.
        nc.scalar.dma_start(out=wv_sb[:, 0:8, :], in_=wvv[:, 0:8, :])
        nc.scalar.dma_start(out=wv_sb[:, 8:16, :], in_=wvv[:, 8:16, :])
        nc.scalar.dma_start(out=wk_sb, in_=wkv)
        for c0, c1 in ((0, 2), (2, 4), (4, 8), (8, 16)):
            nc.scalar.dma_start(out=wq_sb[:, c0:c1, :], in_=wqv[:, c0:c1, :])
        nc.scalar.dma_start(out=msk_sb, in_=msk[:, :, :])
        nc.scalar.dma_start(out=wo_sb, in_=wov)
        # SP queue: xT block 0 progressively; block 1's first group squeezes in
        # before block 0's tail (slack in proj(0)'s consumption) so proj(1)
        # never waits; blocks 2/3 as single deep-queue instructions.
        for c0, c1 in ((0, 1), (1, 2), (2, 4), (4, 8)):
            nc.sync.dma_start(out=xT_sb[:, c0:c1, 0:512],
                              in_=xTv[:, c0:c1, 0:512])
        nc.sync.dma_start(out=xT_sb[:, 0:4, 512:1024], in_=xTv[:, 0:4, 512:1024])
        nc.sync.dma_start(out=xT_sb[:, 8:16, 0:512], in_=xTv[:, 8:16, 0:512])
        for c4 in range(4, HC, 4):
            nc.sync.dma_start(out=xT_sb[:, c4:c4 + 4, 512:1024],
                              in_=xTv[:, c4:c4 + 4, 512:1024])
        nc.sync.dma_start(out=xT_sb[:, :, 1024:1536], in_=xTv[:, :, 1024:1536])
        nc.sync.dma_start(out=xT_sb[:, :, 1536:2048], in_=xTv[:, :, 1536:2048])

        otns = {}

        def proj(t4):
            # v, k first, q heads last: the next phase's first PSUM allocation
            # on the shared tag then waits on an early copy, not the last one.
            tsl = slice(t4 * 512, (t4 + 1) * 512)
            for tt in range(4 * t4, 4 * t4 + 4):
                vp = psum.tile([128, HD], F32, tag="ot", bufs=2, name=f"vp_{tt}")
                for c in range(HC):
                    nc.tensor.matmul(
                        vp,
                        lhsT=xT_sb[:, c, tt * 128:(tt + 1) * 128],
                        rhs=wv_sb[:, c, :],
                        start=(c == 0), stop=(c == HC - 1),
                    )
                nc.vector.tensor_copy(v_sb[:, tt, :], vp)
            kp = psum.tile([128, 512], F32, tag="ot", bufs=2, name=f"kp_{t4}")
            for c in range(HC):
                nc.tensor.matmul(
                    kp,
                    lhsT=wk_sb[:, c, :],
                    rhs=xT_sb[:, c, tsl],
                    start=(c == 0), stop=(c == HC - 1),
                )
            nc.vector.tensor_copy(kt_sb[:, tsl], kp)
            for h in range(G):
                qp = psum.tile([128, 512], F32, tag="ot", bufs=2,
                               name=f"qp_{h}_{t4}")
                for c in range(HC):
                    nc.tensor.matmul(
                        qp,
                        lhsT=wq_sb[:, c, h * HD:(h + 1) * HD],
                        rhs=xT_sb[:, c, tsl],
                        start=(c == 0), stop=(c == HC - 1),
                    )
                nc.vector.tensor_copy(qt_sb[:, h, tsl], qp)

        def attn(qc):
            # Flat software-pipelined stream over (head, key-tile-pair):
            # scores for pair i+1 issue on the PE while the Act engine exps
            # pair i and the PE then runs PV/row-sum of pair i. Uniform across
            # head boundaries, so the last diagonal exp of a head no longer
            # stalls the next head's scores on st-buffer reuse.
            njt = 4 * qc + 4
            items = []
            for h in range(G):
                for p in range(njt // 2):
                    items.append((h, 2 * p, 2 * p == njt - 2))

            def emit_scores(it):
                h, j0, _ = it
                st = psum.tile([128, 1024], F32, tag="st", bufs=2,
                               name=f"st_{qc}_{h}_{j0}")
                pt = ptp.tile([128, 1024], BF16, tag="pt", bufs=8,
                              name=f"pt_{qc}_{h}_{j0}")
                j_a, j_b = j0, j0 + 1
                jr_a, jr_b = j_a - 4 * qc, j_b - 4 * qc  # >= 0 on diagonal
                lo_a = jr_a * 128 if jr_a > 0 else 0
                lo_b = jr_b * 128 if jr_b > 0 else 0
                # diagonal pairs: larger-trim tile goes in half 0 so the
                # single exp starts later (pair at the corner becomes gap-free)
                swap = lo_b > 0
                ha, hb = (1, 0) if swap else (0, 1)
                lor = [(j_a, jr_a, lo_a, ha), (j_b, jr_b, lo_b, hb)]
                for j, jr, lo, hf in lor:
                    nc.tensor.matmul(
                        st[:, hf * 512 + lo:(hf + 1) * 512],
                        lhsT=kt_sb[:, j * 128:(j + 1) * 128],
                        rhs=qt_sb[:, h, qc * 512 + lo:(qc + 1) * 512],
                        start=True, stop=True,
                    )
                lo0 = lo_b if swap else lo_a
                # one exp covering both written halves; any interior gap is
                # stale-but-bounded PSUM whose pt image is never read.
                nc.scalar.activation(pt[:, lo0:], st[:, lo0:],
                                     EXP, scale=float(SCALE))
                for j, jr, lo, hf in lor:
                    if jr >= 0:
                        nc.vector.tensor_mul(
                            pt[:, hf * 512 + lo:(hf + 1) * 512],
                            pt[:, hf * 512 + lo:(hf + 1) * 512],
                            msk_sb[:, jr, lo:])
                return (it, pt, lor)

            def emit_pvls(sc):
                (h, j0, last), pt, lor = sc
                if j0 == 0:
                    state[h] = (
                        psum.tile([128, 512], F32, tag="ot", bufs=2,
                                  name=f"ot_{qc}_{h}"),
                        psum.tile([128, 512], F32, tag="lsd", bufs=2,
                                  name=f"ls_{qc}_{h}"),
                    )
                ot, ls = state[h]
                for j, jr, lo, hf in lor:
                    nc.tensor.matmul(ot[:, lo:], lhsT=v_sb[:, j, :],
                                     rhs=pt[:, hf * 512 + lo:(hf + 1) * 512],
                                     start=(j == 0), stop=(j == njt - 1),
                                     skip_group_check=True)
                for j, jr, lo, hf in lor:
                    nc.tensor.matmul(ls[:, lo:], lhsT=ones_sb,
                                     rhs=pt[:, hf * 512 + lo:(hf + 1) * 512],
                                     start=(j == 0), stop=(j == njt - 1),
                                     skip_group_check=True)
                if last:
                    lnl = vecp.tile([128, 512], F32, tag="lnl", bufs=2,
                                    name=f"lnl_{qc}_{h}")
                    nc.scalar.activation(lnl, ls,
                                         mybir.ActivationFunctionType.Ln)
                    rec = vecp.tile([128, 512], F32, tag="rec", bufs=2,
                                    name=f"rec_{qc}_{h}")
                    nc.scalar.activation(rec, lnl, EXP, scale=-1.0)
                    otn = otnp.tile([128, 512], BF16, tag="otn", bufs=8,
                                    name=f"otn_{qc}_{h}")
                    nc.vector.tensor_mul(otn, ot, rec)
                    otns[h] = otn

            state = {}
            prev = None
            for it in items:
                sc = emit_scores(it)
                if prev is not None:
                    emit_pvls(prev)
                prev = sc
            emit_pvls(prev)

        def oproj(qc, last=False):
            for tt in range(4):
                stage = outp.tile([128, HIDDEN], BF16, tag="stage", bufs=3,
                                  name=f"stage_{qc}_{tt}")
                final = last
                # last phase: per-ec DMAs alternating across both HWDGE
                # queues so the final 2MB drains at double bandwidth (the Act
                # queue is idle then; earlier phases keep it clear for exps)
                for ec in range(4):
                    op = psum.tile([128, 512], F32, tag="ot", bufs=2,
                                   name=f"op_{qc}_{tt}_{ec}")
                    for h in range(G):
                        nc.tensor.matmul(
                            op,
                            lhsT=otns[h][:, tt * 128:(tt + 1) * 128],
                            rhs=wo_sb[:, h, ec * 512:(ec + 1) * 512],
                            start=(h == 0), stop=(h == G - 1),
                        )
                    nc.vector.tensor_copy(stage[:, ec * 512:(ec + 1) * 512], op)
                    if final:
                        # last output tiles: per-ec DMA to shorten the tail
                        r0 = qc * 512 + tt * 128
                        eng = nc.scalar if ec % 2 == 1 else nc.sync
                        eng.dma_start(
                            out=out[r0:r0 + 128, ec * 512:(ec + 1) * 512],
                            in_=stage[:, ec * 512:(ec + 1) * 512])
                if not final:
                    r0 = qc * 512 + tt * 128
                    nc.sync.dma_start(out=out[r0:r0 + 128, :], in_=stage)

        proj(0)
        attn(0)
        proj(1)
        oproj(0)
        attn(1)
        proj(2)
        oproj(1)
        attn(2)
        proj(3)
        oproj(2)
        attn(3)
        oproj(3, last=True)
    return nc


def _masks():
    kl = np.arange(128)[:, None, None]
    jj = np.arange(G)[None, :, None]
    ql = np.arange(512)[None, None, :]
    return (128 * jj + kl <= ql).astype(ml_dtypes.bfloat16)


def kernel(x, w_q, w_kv, w_o):
    global LAST_RESULTS
    if "nc" not in _CACHE:
        _CACHE["nc"] = _build_program()
        _CACHE["msk"] = _masks()
    nc = _CACHE["nc"]
    bf = ml_dtypes.bfloat16
    x = np.asarray(x, dtype=np.float32)
    w_q = np.asarray(w_q, dtype=np.float32)
    w_kv = np.asarray(w_kv, dtype=np.float32)
    w_o = np.asarray(w_o, dtype=np.float32)

    in_maps = []
    for c in range(NCORES):
        b, g = c // 4, c % 4
        in_maps.append({
            "xT": np.ascontiguousarray(x[b].T).astype(bf),
            "wq": np.ascontiguousarray(w_q[512 * g:512 * (g + 1), :].T).astype(bf),
            "wk": np.ascontiguousarray(w_kv[128 * g:128 * (g + 1), :].T).astype(bf),
            "wv": np.ascontiguousarray(
                w_kv[512 + 128 * g:512 + 128 * (g + 1), :].T).astype(bf),
            "wo": np.ascontiguousarray(w_o[:, 512 * g:512 * (g + 1)].T).astype(bf),
            "msk": _CACHE["msk"],
        })

    res = run_bass_kernel_spmd(nc, in_maps, core_ids=list(range(NCORES)))
    LAST_RESULTS = res
    outs = res.results
    o = [outs[c]["out"].astype(np.float32) for c in range(NCORES)]
    out = np.stack([o[0] + o[1] + o[2] + o[3], o[4] + o[5] + o[6] + o[7]])
    return out
